# revision 21
# baseline (speedup 1.0000x reference)
"""BlurPool3D Trainium2 kernel.

Depthwise 3x3x3 separable (rank-1) blur, stride 2, pad 1 on
x[2, 64, 64, 96, 96] f32 -> y[2, 64, 32, 48, 48] f32.

Strategy (8 NeuronCores, SPMD, DMA-roofline oriented):
  - Shard the 128 (n, c) pairs across cores: 16 per core, 8 blocks of
    2 channels. Channels are independent in a depthwise conv -> no
    halo, no collectives.
  - Per block, SBUF partitions = (2 nc x 64 d) = 128, free = spatial.
    The full D axis lives on partitions, so the D-tap contraction is a
    matmul with a block-diagonal band lhsT; the 3 H taps are 3
    accumulating matmuls reading h-shifted rows at stride 2. D/H edges
    are handled by the band matrix / a zeroed bf16 pad row.
  - The W contraction runs AFTER the matmul, on the (D/2, H/2)-
    downsampled data -> 4x less VectorE work than a pre-matmul
    W-pass, so every engine sits well under the DMA roofline (the
    binding constraint: 42.5 MB/core of HBM traffic at ~360-420 GB/s;
    NC pairs share a 716 GB/s HBM stack).
  - One pipeline stage per engine queue, so no stage is ever
    head-of-line blocked behind another block's work: input DMAs on
    the HWDGE sync ring (f32, 3 x 1.57 MB chunks per block); f32 ->
    bf16 casts on VectorE (2 elem/cyc single-src mode), issued one
    block ahead of the W-pass ops; PSUM -> SBUF drains on ScalarE;
    the 3-tap W combine on VectorE; output DMA issues on GpSimd
    (SWDGE).
  - Matmuls run in bf16 (exact for the binomial taps), accumulating
    in fp32 PSUM over 3 H taps per 5/5/5/5/4-row chunk; the two
    h-halves map to PE column groups 0/1 (tile_position) writing PSUM
    partitions 0-63/64-127. A dense dummy-matmul burst at t=0 spans
    the framework preamble so the PE HAM clock-gate is released
    (2.4 GHz) before the first real matmul; a cold PE (1.2 GHz)
    cannot keep up with the DMA stream and drags the pipeline into a
    throttled slow mode.
  - The last block streams input in 16-row pieces and runs the W
    combine + output per PSUM chunk, keeping the serial tail after
    the final input bytes to a few us.
"""

import os
import sys

for _p in ("/opt/trn_rl_repo",):
    if _p not in sys.path and os.path.isdir(_p):
        sys.path.insert(0, _p)

import numpy as np

N, C, D, H, W = 2, 64, 64, 96, 96
DO, HO, WO = 32, 48, 48
NCORES = 8
NC_PER_CORE = (N * C) // NCORES  # 16
BLOCKS = NC_PER_CORE // 2  # 8 blocks of 2 channels each

_PROGRAM_CACHE = {}


def _rank1_factors(filt):
    """Per-channel rank-1 factorization filt[c,0] = outer(d, h, w)."""
    dvec = np.empty((C, 3), np.float64)
    hvec = np.empty((C, 3), np.float64)
    wvec = np.empty((C, 3), np.float64)
    for c in range(C):
        T = filt[c, 0].astype(np.float64)
        idx = np.unravel_index(np.argmax(np.abs(T)), T.shape)
        i0, j0, k0 = idx
        piv = T[i0, j0, k0]
        if piv == 0.0:
            dvec[c] = hvec[c] = wvec[c] = 0.0
            continue
        dvec[c] = T[:, j0, k0]
        hvec[c] = T[i0, :, k0] / piv
        wvec[c] = T[i0, j0, :] / piv
        recon = np.einsum("i,j,k->ijk", dvec[c], hvec[c], wvec[c])
        resid = np.abs(recon - T).max()
        if resid > 1e-6 * max(np.abs(T).max(), 1e-30):
            raise ValueError(f"filter channel {c} is not rank-1 (resid {resid})")
    return dvec, hvec, wvec


def _build_program(uniform, tt2):
    import concourse.bacc as bacc
    import concourse.mybir as mybir
    from concourse import tile

    dt = mybir.dt
    nc = bacc.Bacc("TRN2", target_bir_lowering=False, debug=False,
                   num_devices=NCORES)

    nbm = 1 if uniform else BLOCKS
    x = nc.dram_tensor("x", [NC_PER_CORE, D, H * W], dt.float32,
                       kind="ExternalInput")
    bmat = nc.dram_tensor("bmat", [128, nbm * 3 * 64], dt.bfloat16,
                          kind="ExternalInput")
    wtaps = nc.dram_tensor("wtaps", [128, 2 * BLOCKS], dt.float32,
                           kind="ExternalInput")
    y = nc.dram_tensor("y", [NC_PER_CORE, DO, HO * WO], dt.float32,
                       kind="ExternalOutput")

    # Per block: 48 output h-rows as two halves g (PE column groups /
    # PSUM partition halves), 5 PSUM chunks of 5/5/5/5/4 output rows
    # each (5 rows x 96 = 480 f32 just fits a 2 KB PSUM bank); every
    # chunk accumulates 3 H-tap matmuls per half. Bigger chunks mean
    # fewer matmul instructions, which keeps the cold-clock (HAM
    # K=4/8) PE time per block under the DMA per-block budget.
    CSTART = (0, 5, 10, 15, 20)
    CSIZE = (5, 5, 5, 5, 4)
    NCHUNK = 5

    with tile.TileContext(nc) as tc:
        with tc.tile_pool(name="const", bufs=1) as cpool, \
             tc.tile_pool(name="x32", bufs=7) as x32pool, \
             tc.tile_pool(name="xb", bufs=3) as xbpool, \
             tc.tile_pool(name="zt", bufs=3) as ztpool, \
             tc.tile_pool(name="op", bufs=3) as opool, \
             tc.tile_pool(name="ps", bufs=7, space="PSUM") as pspool:
            bt = cpool.tile([128, nbm * 3 * 64], dt.bfloat16)
            wt = cpool.tile([128, 2 * BLOCKS], dt.float32)
            nc.scalar.dma_start(bt[:], bmat[:])
            nc.scalar.dma_start(wt[:], wtaps[:])

            # PE HAM warm-up: the PE clock-gates to 1.2 GHz until it
            # sees ~3.4 us of sustained activity, and the first real
            # matmul can't start until the preamble + first input DMA
            # land (~12 us in). A dense burst of dummy matmuls on a
            # zeroed scratch tile spans that window so the real work
            # starts (and stays) at 2.4 GHz; cold, the PE per-block
            # time exceeds the DMA per-block budget and the whole
            # pipeline settles into a throttled slow mode.
            wm_s = cpool.tile([128, 64], dt.bfloat16, name="wm_s")
            nc.vector.memset(wm_s[:], 0.0)
            wmp = pspool.tile([128, 64], dt.float32, tag="wmps", bufs=1,
                              name="wmp")
            for _ in range(112):
                nc.tensor.matmul(wmp[:64, :], wm_s[:], wm_s[:],
                                 start=True, stop=True)

            def issue_input(b):
                """Input DMAs + f32->bf16 casts for block b.

                The bf16 tile has a zeroed pad row 0 (x row -1), rows
                1..96 = x rows 0..95. The last block streams in finer
                pieces so the serial tail after the final bytes stays
                short.
                """
                last = b == BLOCKS - 1
                src = x[2 * b:2 * b + 2].rearrange("a d f -> (a d) f")
                src = src.rearrange("p (h w) -> p h w", h=H)
                rsub = 16 if last else 32
                xb = xbpool.tile([128, H + 1, W], dt.bfloat16, tag="xb",
                                 name="xb")
                nc.gpsimd.memset(xb[:, 0:1, :], 0.0)
                for s in range(H // rsub):
                    if last:
                        x32 = x32pool.tile([128, rsub, W], dt.float32,
                                           tag="x32l", bufs=3, name="x32l")
                    else:
                        x32 = x32pool.tile([128, rsub, W], dt.float32,
                                           tag="x32", name="x32")
                    nc.sync.dma_start(x32[:],
                                      src[:, s * rsub:(s + 1) * rsub, :])
                    nc.vector.tensor_copy(
                        xb[:, 1 + s * rsub:1 + (s + 1) * rsub, :], x32[:])
                return xb

            def compute_block(b, xb):
                last = b == BLOCKS - 1
                bcol = 0 if uniform else b * 3 * 64
                r1 = wt[:, 2 * b:2 * b + 1]
                r2 = wt[:, 2 * b + 1:2 * b + 2]

                # Fused H+D matmuls straight off the bf16 input: chunk
                # (g, ci) covers output h' rows g*24+4*ci..+3; tap k
                # reads xb rows 2*(24g+4ci)+k..+6 stride 2 (row 0 is
                # the h pad). The two halves map to PE column groups
                # 0/1 writing PSUM partitions 0-63 / 64-127.
                zt = ztpool.tile([128, 24, W], dt.float32, tag="zt")
                ot = opool.tile([128, 24, WO], dt.float32, tag="ot")
                for ci in range(NCHUNK):
                    c0, cn = CSTART[ci], CSIZE[ci]
                    psv = pspool.tile([128, 5 * W], dt.float32, tag="ps",
                                      name="ps")
                    for g in range(2):
                        row0 = 2 * (24 * g + c0)
                        for k in range(3):
                            lhsT = bt[:, bcol + k * 64:bcol + (k + 1) * 64]
                            rhs = xb[:, row0 + k:row0 + k + 2 * cn - 1:2, :]
                            nc.tensor.matmul(
                                psv[64:, :cn * W] if g else psv[:64, :cn * W],
                                lhsT, rhs,
                                start=(k == 0), stop=(k == 2),
                                tile_position=(0, 64 * g) if g else None)
                    # PSUM -> SBUF drain on ScalarE.
                    nc.scalar.copy(
                        zt[:, c0:c0 + cn, :], psv[:, :cn * W])

                    # W-pass on VectorE once its zt rows land (per
                    # 12-row half normally; per 4-row chunk on the
                    # last block to keep the tail short):
                    # p[w'] = z[2w'-1] + r1*z[2w'] + r2*z[2w'+1]
                    # (w'=0 left tap is the zero pad -> edge op).
                    if last:
                        r0, rn = c0, cn
                    elif ci == 2:
                        r0, rn = 0, 12
                    elif ci == NCHUNK - 1:
                        r0, rn = 12, 12
                    else:
                        continue
                    rows = slice(r0, r0 + rn)
                    nc.vector.scalar_tensor_tensor(
                        ot[:, rows, 1:WO],
                        zt[:, rows, 2:2 * WO - 1:2], r1,
                        zt[:, rows, 1:2 * WO - 2:2],
                        mybir.AluOpType.mult, mybir.AluOpType.add)
                    if tt2:
                        nc.vector.tensor_add(
                            ot[:, rows, 1:WO],
                            zt[:, rows, 3:2 * WO:2],
                            ot[:, rows, 1:WO])
                        nc.vector.scalar_tensor_tensor(
                            ot[:, rows, 0:1],
                            zt[:, rows, 0:1], r1, zt[:, rows, 1:2],
                            mybir.AluOpType.mult, mybir.AluOpType.add)
                    else:
                        nc.vector.scalar_tensor_tensor(
                            ot[:, rows, 1:WO],
                            zt[:, rows, 3:2 * WO:2], r2,
                            ot[:, rows, 1:WO],
                            mybir.AluOpType.mult, mybir.AluOpType.add)
                        nc.vector.tensor_scalar(
                            ot[:, rows, 0:1], zt[:, rows, 1:2], r2,
                            None, mybir.AluOpType.mult)
                        nc.vector.scalar_tensor_tensor(
                            ot[:, rows, 0:1],
                            zt[:, rows, 0:1], r1, ot[:, rows, 0:1],
                            mybir.AluOpType.mult, mybir.AluOpType.add)
                    # Ship each (g, row-range) as soon as it's done.
                    # Output issues ride SWDGE (gpsimd) so a W-pass
                    # running late never head-of-line blocks the next
                    # block's drains on the ACT queue; h' = g*24 + r.
                    for g in range(2):
                        dst = y[2 * b:2 * b + 2, :,
                                (g * 24 + r0) * WO:(g * 24 + r0 + rn) * WO]
                        dst = dst.rearrange("a d f -> (a d) f")
                        nc.gpsimd.dma_start(
                            dst, ot[g * 64:(g + 1) * 64,
                                    r0:r0 + rn, :])

            # Software-pipelined by one block: block b+1's casts are
            # issued to the VectorE queue BEFORE block b's W-pass, so
            # the PE's next-block matmuls are never head-of-line
            # blocked behind drain -> W -> cast at a block boundary
            # (that bubble is what lets the PE HAM throttle re-engage).
            xb_prev = issue_input(0)
            for b in range(1, BLOCKS):
                xb_cur = issue_input(b)
                compute_block(b - 1, xb_prev)
                xb_prev = xb_cur
            compute_block(BLOCKS - 1, xb_prev)
    nc.compile()
    return nc


def kernel(x, filt):
    x = np.ascontiguousarray(np.asarray(x, dtype=np.float32))
    filt = np.asarray(filt, dtype=np.float32)
    assert x.shape == (N, C, D, H, W), x.shape

    import ml_dtypes
    from concourse.bass_utils import run_bass_kernel_spmd

    dvec, hvec, wvec = _rank1_factors(filt)
    # W pivot (left tap w0) folded into the matmul matrices.
    w0 = wvec[:, 0].copy()
    safe = np.abs(w0) > 1e-30
    if not safe.all():
        raise ValueError("W-tap pivot is zero; unsupported filter")
    r1 = wvec[:, 1] / w0
    r2 = wvec[:, 2] / w0

    uniform = bool(np.all(filt == filt[:1]))
    xr = x.reshape(N * C, D, H * W)

    in_maps = []
    for core in range(NCORES):
        chans = (np.arange(NC_PER_CORE) + core * NC_PER_CORE) % C  # local->c
        wt = np.empty((128, 2 * BLOCKS), np.float32)
        bm = np.zeros((128, (1 if uniform else BLOCKS) * 3 * 64), np.float64)
        for b in range(BLOCKS):
            for ncl in range(2):
                c = chans[2 * b + ncl]
                wt[ncl * 64:(ncl + 1) * 64, 2 * b + 0] = r1[c]
                wt[ncl * 64:(ncl + 1) * 64, 2 * b + 1] = r2[c]
                if uniform and b > 0:
                    continue
                # band matrix rows (ncl*64 + d), cols (ncl*32 + d'),
                # one 64-col group per H tap k, scaled by hvec[k] and
                # the W pivot w0.
                for k in range(3):
                    col0 = (b * 3 + k) * 64 + ncl * 32
                    for dp in range(DO):
                        for delta in range(3):
                            d = 2 * dp - 1 + delta
                            if 0 <= d < D:
                                bm[ncl * 64 + d, col0 + dp] = (
                                    dvec[c, delta] * hvec[c, k] * w0[c])
        in_maps.append({
            "x": np.ascontiguousarray(
                xr[core * NC_PER_CORE:(core + 1) * NC_PER_CORE]),
            "bmat": bm.astype(ml_dtypes.bfloat16),
            "wtaps": wt,
        })

    tt2 = bool(np.allclose(r2, 1.0, rtol=0, atol=0))
    key = ("prog", uniform, tt2)
    if key not in _PROGRAM_CACHE:
        _PROGRAM_CACHE[key] = _build_program(uniform, tt2)
    nc = _PROGRAM_CACHE[key]

    trace = bool(int(os.environ.get("BLURPOOL_TRACE", "0")))
    kwargs = {}
    if trace and os.environ.get("BLURPOOL_TRACE_DIR"):
        kwargs["tmpdir"] = os.environ["BLURPOOL_TRACE_DIR"]
    res = run_bass_kernel_spmd(nc, in_maps, core_ids=list(range(NCORES)),
                               trace=trace, **kwargs)
    if trace:
        kernel.last_result = res

    out = np.concatenate([r["y"].reshape(NC_PER_CORE, DO, HO, WO)
                          for r in res.results], axis=0)
    return np.ascontiguousarray(out.reshape(N, C, DO, HO, WO))
